# revision 3
# baseline (speedup 1.0000x reference)
"""Trainium2 Bass kernel for CWL2GCNLayer (WL2 GNN message passing).

reference:
    XW = X @ W; XW_prop = X @ W_prop; XW_back = X @ W_back
    S = relu(XW_prop[ref_a] + XW_prop[ref_b] + b_prop)        # [M, 64]
    conv = segment_sum(S, backref, num_segments=N)            # [N, 64]
    out = relu(XW + XW_back * conv + b)

Strategy (8 NeuronCores, SPMD single program):
  - Partition the M pair-entries by owner core of backref (8 row slices).
  - Each core computes the full gather table T = X @ W_prop + b_prop/2 on
    device (bias folded via an appended ones-row in X^T / bias row in W),
    stored in its DRAM in a core-rotated row order so that the core's own
    row slice occupies local rows [0, ROWS_PER_CORE) -- keeps the program
    identical across cores (SPMD).
  - Entries sorted by backref -> 128-row windows; each window has K chunks
    of 128 entries (host-padded).  Per window: indirect-DMA gather of a/b
    rows, S = relu(ga + gb) (bias already folded, x2), one-hot Q built with
    a DVE is_equal against an iota constant, and K PE matmuls
    psum += Q_c^T @ S_c accumulate the segment sum in PSUM.
  - Combine relu(XW + XW_back*conv + b) from SBUF-resident projections of
    the core's own row slice; DMA out.
"""
import numpy as np

from concourse import bass, mybir, bacc, tile
from concourse.bass import IndirectOffsetOnAxis

N_CORES = 8
D = 64
CH = 128                      # entries per chunk / rows per window

_cache = {}


# ----------------------------------------------------------------------------
# host-side prep
# ----------------------------------------------------------------------------

def _host_prep(X, ref_a, ref_b, backref, W, W_back, W_prop, b, b_prop):
    N = X.shape[0]
    rpc = -(-N // (N_CORES * CH)) * CH          # rows per core, mult of 128
    nw = rpc // CH                               # windows per core
    nt_pad = rpc * N_CORES                       # padded table rows

    order = np.argsort(backref, kind="stable")
    sb = np.asarray(backref)[order].astype(np.int64)
    sa = np.asarray(ref_a)[order].astype(np.int64)
    sbb = np.asarray(ref_b)[order].astype(np.int64)

    core_starts = np.searchsorted(sb, np.arange(N_CORES + 1) * rpc)
    win_global = sb // CH
    cnt = np.bincount(win_global, minlength=N_CORES * nw)
    K = max(1, int(-(-cnt.max() // CH)))

    E = nw * K * CH
    ncol = nw * K

    X_pad = np.zeros((nt_pad, D + 1), np.float32)
    X_pad[:N, :D] = np.asarray(X, np.float32)
    X_pad[:N, D] = 1.0                           # ones column for bias row

    Wf = np.asarray(W, np.float32)
    Wbk = np.asarray(W_back, np.float32)
    Wp = np.asarray(W_prop, np.float32)
    bf = np.asarray(b, np.float32)
    bpf = np.asarray(b_prop, np.float32)

    w_main = np.concatenate([Wf, bf[None, :]], axis=0)            # [65, 64]
    w_back = np.concatenate([Wbk, np.zeros((1, D), np.float32)], axis=0)
    w_prop = np.concatenate([Wp, 0.5 * bpf[None, :]], axis=0)

    iota = np.broadcast_to(
        np.tile(np.arange(CH, dtype=np.float32), K), (CH, K * CH)
    ).copy()

    in_maps = []
    for c in range(N_CORES):
        seg = slice(core_starts[c], core_starts[c + 1])
        seg_b = sb[seg]
        seg_a = sa[seg]
        seg_bb = sbb[seg]
        w_local = (seg_b - c * rpc) // CH
        win_starts = np.searchsorted(seg_b, c * rpc + np.arange(nw) * CH)
        slot = np.arange(len(seg_b)) - win_starts[w_local]
        dest = w_local * (K * CH) + slot

        ia = np.zeros(E, np.int32)
        ib = np.zeros(E, np.int32)
        lbr = np.full(E, -1.0, np.float32)
        ia[dest] = ((seg_a - c * rpc) % nt_pad).astype(np.int32)
        ib[dest] = ((seg_bb - c * rpc) % nt_pad).astype(np.int32)
        lbr[dest] = (seg_b - (c * rpc + w_local * CH)).astype(np.float32)

        xT = np.ascontiguousarray(
            np.roll(X_pad, -c * rpc, axis=0).T
        )                                         # [65, nt_pad]

        in_maps.append({
            "xT": xT,
            "w_main": w_main,
            "w_back": w_back,
            "w_prop": w_prop,
            "idxa": np.ascontiguousarray(ia.reshape(ncol, CH).T),
            "idxb": np.ascontiguousarray(ib.reshape(ncol, CH).T),
            "lbr": np.ascontiguousarray(lbr.reshape(ncol, CH).T),
            "iota": iota,
        })

    cfg = dict(N=N, rpc=rpc, nw=nw, nt_pad=nt_pad, K=K)
    return in_maps, cfg


# ----------------------------------------------------------------------------
# device program
# ----------------------------------------------------------------------------

def _build_program(cfg):
    rpc, nw, nt_pad, K = cfg["rpc"], cfg["nw"], cfg["nt_pad"], cfg["K"]
    ncol = nw * K
    ntile = nt_pad // CH                 # 128-row tiles in the table
    TB = 4                               # table tiles per phase-1 batch
    assert ntile % TB == 0 and nw % TB == 0
    nbatch = ntile // TB
    own_batches = nw // TB               # first own_batches cover own slice

    f32 = mybir.dt.float32
    i32 = mybir.dt.int32

    nc = bacc.Bacc("TRN2", target_bir_lowering=False, debug=False,
                   num_devices=N_CORES)

    xT = nc.dram_tensor("xT", [D + 1, nt_pad], f32, kind="ExternalInput").ap()
    w_main = nc.dram_tensor("w_main", [D + 1, D], f32, kind="ExternalInput").ap()
    w_back = nc.dram_tensor("w_back", [D + 1, D], f32, kind="ExternalInput").ap()
    w_prop = nc.dram_tensor("w_prop", [D + 1, D], f32, kind="ExternalInput").ap()
    idxa = nc.dram_tensor("idxa", [CH, ncol], i32, kind="ExternalInput").ap()
    idxb = nc.dram_tensor("idxb", [CH, ncol], i32, kind="ExternalInput").ap()
    lbr = nc.dram_tensor("lbr", [CH, ncol], f32, kind="ExternalInput").ap()
    iota = nc.dram_tensor("iota", [CH, K * CH], f32, kind="ExternalInput").ap()
    table = nc.dram_tensor("table", [nt_pad, D], f32).ap()
    outp = nc.dram_tensor("out", [rpc, D], f32, kind="ExternalOutput").ap()

    with tile.TileContext(nc) as tc:
        with (
            tc.tile_pool(name="wp", bufs=1) as wp,
            tc.tile_pool(name="xp", bufs=3) as xp,
            tc.tile_pool(name="pers", bufs=1) as pers,
            tc.tile_pool(name="stg", bufs=3) as stg,
            tc.tile_pool(name="idxp", bufs=1) as idxp,
            tc.tile_pool(name="gp", bufs=3) as gp,
            tc.tile_pool(name="sp", bufs=2) as sp,
            tc.tile_pool(name="qp", bufs=2) as qp,
            tc.tile_pool(name="op", bufs=3) as op,
            tc.tile_pool(name="ps1", bufs=4, space="PSUM") as ps1,
            tc.tile_pool(name="ps2", bufs=3, space="PSUM") as ps2,
        ):
            wm_t = wp.tile([D + 1, D], f32)
            wb_t = wp.tile([D + 1, D], f32)
            wpr_t = wp.tile([D + 1, D], f32)
            nc.sync.dma_start(out=wm_t[:], in_=w_main[:])
            nc.sync.dma_start(out=wb_t[:], in_=w_back[:])
            nc.sync.dma_start(out=wpr_t[:], in_=w_prop[:])

            ia_t = idxp.tile([CH, ncol], i32)
            ib_t = idxp.tile([CH, ncol], i32)
            lbr_t = idxp.tile([CH, ncol], f32)
            iot_t = idxp.tile([CH, K * CH], f32)
            nc.sync.dma_start(out=ia_t[:], in_=idxa[:])
            nc.sync.dma_start(out=ib_t[:], in_=idxb[:])
            nc.sync.dma_start(out=lbr_t[:], in_=lbr[:])
            nc.sync.dma_start(out=iot_t[:], in_=iota[:])

            xw_s = pers.tile([CH, nw * D], f32)    # XW + b, own slice
            xwb_s = pers.tile([CH, nw * D], f32)   # XW_back, own slice

            # ---------------- phase 1: build table (+ own projections) ----
            for bt in range(nbatch):
                c0 = bt * TB * CH
                xb = xp.tile([D + 1, TB * CH], f32, tag="xb")
                nc.sync.dma_start(out=xb[:], in_=xT[:, c0:c0 + TB * CH])
                ps = ps1.tile([CH, TB * D], f32, tag="psA", space="PSUM")
                for k in range(TB):
                    nc.tensor.matmul(
                        out=ps[:, k * D:(k + 1) * D],
                        lhsT=xb[:, k * CH:(k + 1) * CH],
                        rhs=wpr_t[:],
                        start=True, stop=True,
                    )
                st = stg.tile([CH, TB * D], f32, tag="st")
                nc.scalar.copy(st[:], ps[:])
                nc.sync.dma_start(
                    out=table[bt * TB * CH:(bt + 1) * TB * CH, :].rearrange(
                        "(k p) d -> p k d", p=CH),
                    in_=st[:].rearrange("p (k d) -> p k d", d=D),
                )
                if bt < own_batches:
                    psb = ps1.tile([CH, TB * D], f32, tag="psA", space="PSUM")
                    psc = ps1.tile([CH, TB * D], f32, tag="psA", space="PSUM")
                    for k in range(TB):
                        nc.tensor.matmul(
                            out=psb[:, k * D:(k + 1) * D],
                            lhsT=xb[:, k * CH:(k + 1) * CH],
                            rhs=wm_t[:], start=True, stop=True,
                        )
                        nc.tensor.matmul(
                            out=psc[:, k * D:(k + 1) * D],
                            lhsT=xb[:, k * CH:(k + 1) * CH],
                            rhs=wb_t[:], start=True, stop=True,
                        )
                    nc.scalar.copy(xw_s[:, bt * TB * D:(bt + 1) * TB * D], psb[:])
                    nc.scalar.copy(xwb_s[:, bt * TB * D:(bt + 1) * TB * D], psc[:])

            # ---------------- phase 2: gather / segsum / combine ----------
            for w in range(nw):
                ga = gp.tile([CH, K * D], f32, tag="ga")
                gb = gp.tile([CH, K * D], f32, tag="gb")
                # HW indirect DMA honours only ONE offset per partition:
                # one [128,1]-offset gather per 128-entry chunk.
                for k in range(K):
                    nc.gpsimd.indirect_dma_start(
                        out=ga[:, k * D:(k + 1) * D], out_offset=None,
                        in_=table[:],
                        in_offset=IndirectOffsetOnAxis(
                            ap=ia_t[:, w * K + k:w * K + k + 1], axis=0),
                    )
                    nc.gpsimd.indirect_dma_start(
                        out=gb[:, k * D:(k + 1) * D], out_offset=None,
                        in_=table[:],
                        in_offset=IndirectOffsetOnAxis(
                            ap=ib_t[:, w * K + k:w * K + k + 1], axis=0),
                    )
                s = sp.tile([CH, K * D], f32, tag="s")
                nc.vector.tensor_add(s[:], ga[:], gb[:])
                nc.scalar.activation(s[:], s[:],
                                     mybir.ActivationFunctionType.Relu)
                q = qp.tile([CH, K * CH], f32, tag="q")
                nc.vector.tensor_tensor(
                    out=q[:].rearrange("p (k r) -> p k r", r=CH),
                    in0=lbr_t[:, w * K:(w + 1) * K].to_broadcast([CH, K, CH]),
                    in1=iot_t[:].rearrange("p (k r) -> p k r", r=CH),
                    op=mybir.AluOpType.is_equal,
                )
                cps = ps2.tile([CH, D], f32, tag="cps", space="PSUM")
                for k in range(K):
                    nc.tensor.matmul(
                        out=cps[:],
                        lhsT=q[:, k * CH:(k + 1) * CH],
                        rhs=s[:, k * D:(k + 1) * D],
                        start=(k == 0), stop=(k == K - 1),
                    )
                t2 = op.tile([CH, D], f32, tag="t2")
                nc.vector.tensor_mul(t2[:], xwb_s[:, w * D:(w + 1) * D], cps[:])
                t3 = op.tile([CH, D], f32, tag="t3")
                nc.vector.tensor_add(t3[:], t2[:], xw_s[:, w * D:(w + 1) * D])
                o = op.tile([CH, D], f32, tag="o")
                nc.scalar.activation(o[:], t3[:],
                                     mybir.ActivationFunctionType.Relu)
                nc.sync.dma_start(out=outp[w * CH:(w + 1) * CH, :], in_=o[:])

    nc.compile()
    return nc


# ----------------------------------------------------------------------------
# SPMD runner (device-resident inputs, PJRT under axon)
# ----------------------------------------------------------------------------

class SpmdRunner:
    def __init__(self, nc, n_cores=N_CORES):
        import jax
        from jax.sharding import Mesh, PartitionSpec
        from jax.experimental.shard_map import shard_map
        from concourse.bass2jax import (
            install_neuronx_cc_hook, _bass_exec_p, partition_id_tensor)

        install_neuronx_cc_hook()
        self.jax = jax
        self.nc = nc
        self.n_cores = n_cores
        partition_name = (nc.partition_id_tensor.name
                          if nc.partition_id_tensor else None)

        in_names, out_names, out_avals, zero_shapes = [], [], [], []
        for alloc in nc.m.functions[0].allocations:
            if not isinstance(alloc, mybir.MemoryLocationSet):
                continue
            name = alloc.memorylocations[0].name
            if alloc.kind == "ExternalInput":
                if name != partition_name and (
                        nc.dbg_addr is None or name != nc.dbg_addr.name):
                    in_names.append(name)
            elif alloc.kind == "ExternalOutput":
                out_names.append(name)
                shape = tuple(alloc.tensor_shape)
                dtype = mybir.dt.np(alloc.dtype)
                out_avals.append(jax.core.ShapedArray(shape, dtype))
                zero_shapes.append((shape, dtype))
        self.in_names, self.out_names = in_names, out_names
        self.out_avals, self.zero_shapes = out_avals, zero_shapes
        n_params, n_outs = len(in_names), len(out_names)

        all_in_names = list(in_names) + list(out_names)
        dbg_name = nc.dbg_addr.name if nc.dbg_addr is not None else None
        if dbg_name is not None:
            all_in_names.append(dbg_name)
        if partition_name is not None:
            all_in_names.append(partition_name)

        def _body(*args):
            operands = list(args)
            if dbg_name is not None:
                operands.append(jax.numpy.zeros((1, 2), jax.numpy.uint32))
            if partition_name is not None:
                operands.append(partition_id_tensor())
            outs = _bass_exec_p.bind(
                *operands,
                out_avals=tuple(out_avals),
                in_names=tuple(all_in_names),
                out_names=tuple(out_names),
                lowering_input_output_aliases=(),
                sim_require_finite=True,
                sim_require_nnan=True,
                nc=nc,
            )
            return tuple(outs)

        devices = jax.devices()[:n_cores]
        self.mesh = Mesh(np.asarray(devices), ("core",))
        self.pspec = PartitionSpec("core")
        in_specs = (self.pspec,) * (n_params + n_outs)
        out_specs = (self.pspec,) * n_outs
        self.fn = jax.jit(
            shard_map(_body, mesh=self.mesh, in_specs=in_specs,
                      out_specs=out_specs, check_rep=False),
            donate_argnums=tuple(range(n_params, n_params + n_outs)),
            keep_unused=True,
        )

    def put_inputs(self, in_maps):
        sharding = self.jax.sharding.NamedSharding(self.mesh, self.pspec)
        return [
            self.jax.device_put(
                np.concatenate([np.asarray(m[name]) for m in in_maps], axis=0),
                sharding)
            for name in self.in_names
        ]

    def _zeros(self):
        sharding = self.jax.sharding.NamedSharding(self.mesh, self.pspec)
        return [
            self.jax.device_put(
                np.zeros((self.n_cores * s[0], *s[1:]), d), sharding)
            for (s, d) in self.zero_shapes
        ]

    def run(self, dev_in):
        outs = self.fn(*dev_in, *self._zeros())
        self.jax.block_until_ready(outs)
        return outs

    def results(self, outs):
        res = []
        for c in range(self.n_cores):
            d = {}
            for i, name in enumerate(self.out_names):
                shp = self.out_avals[i].shape
                d[name] = np.asarray(outs[i]).reshape(
                    self.n_cores, *shp)[c]
            res.append(d)
        return res


# ----------------------------------------------------------------------------
# entry point
# ----------------------------------------------------------------------------

def kernel(X, ref_a, ref_b, backref, e_map, v_count, W, W_back, W_prop,
           b, b_prop, **_unused):
    X = np.asarray(X)
    in_maps, cfg = _host_prep(X, ref_a, ref_b, backref,
                              W, W_back, W_prop, b, b_prop)
    key = (cfg["N"], cfg["rpc"], cfg["K"])
    if key not in _cache:
        nc = _build_program(cfg)
        _cache[key] = SpmdRunner(nc)
    runner = _cache[key]
    dev_in = runner.put_inputs(in_maps)
    outs = runner.run(dev_in)
    res = runner.results(outs)
    full = np.concatenate([res[c]["out"] for c in range(N_CORES)], axis=0)
    return full[:cfg["N"]].astype(np.float32)


# revision 5
# speedup vs baseline: 90.4797x; 90.4797x over previous
"""Trainium2 Bass kernel for CWL2GCNLayer (WL2 GNN message passing).

reference:
    XW = X @ W; XW_prop = X @ W_prop; XW_back = X @ W_back
    S = relu(XW_prop[ref_a] + XW_prop[ref_b] + b_prop)        # [M, 64]
    conv = segment_sum(S, backref, num_segments=N)            # [N, 64]
    out = relu(XW + XW_back * conv + b)

Strategy (8 NeuronCores, SPMD single program):
  - Partition the M pair-entries by owner core of backref (8 row slices).
  - Each core computes the full gather table T = X @ W_prop + b_prop/2 on
    device (bias folded via an appended ones-row in X^T / bias row in W),
    stored in its DRAM in a core-rotated row order so that the core's own
    row slice occupies local rows [0, ROWS_PER_CORE) -- keeps the program
    identical across cores (SPMD).
  - Entries sorted by backref -> 128-row windows; each window has K chunks
    of 128 entries (host-padded).  Per window: indirect-DMA gather of a/b
    rows, S = relu(ga + gb) (bias already folded, x2), one-hot Q built with
    a DVE is_equal against an iota constant, and K PE matmuls
    psum += Q_c^T @ S_c accumulate the segment sum in PSUM.
  - Combine relu(XW + XW_back*conv + b) from SBUF-resident projections of
    the core's own row slice; DMA out.
"""
import numpy as np

from concourse import bass, mybir, bacc, tile
from concourse.bass import IndirectOffsetOnAxis

N_CORES = 8
D = 64
CH = 128                      # entries per chunk / rows per window

_cache = {}


# ----------------------------------------------------------------------------
# host-side prep
# ----------------------------------------------------------------------------

def _host_prep(X, ref_a, ref_b, backref, W, W_back, W_prop, b, b_prop):
    N = X.shape[0]
    rpc = -(-N // (N_CORES * CH)) * CH          # rows per core, mult of 128
    nw = rpc // CH                               # windows per core
    nt_pad = rpc * N_CORES                       # padded table rows

    order = np.argsort(backref, kind="stable")
    sb = np.asarray(backref)[order].astype(np.int64)
    sa = np.asarray(ref_a)[order].astype(np.int64)
    sbb = np.asarray(ref_b)[order].astype(np.int64)

    core_starts = np.searchsorted(sb, np.arange(N_CORES + 1) * rpc)
    win_global = sb // CH
    cnt = np.bincount(win_global, minlength=N_CORES * nw)
    K = max(1, int(-(-cnt.max() // CH)))

    E = nw * K * CH
    ncol = nw * K

    X_pad = np.zeros((nt_pad, D + 1), np.float32)
    X_pad[:N, :D] = np.asarray(X, np.float32)
    X_pad[:N, D] = 1.0                           # ones column for bias row

    Wf = np.asarray(W, np.float32)
    Wbk = np.asarray(W_back, np.float32)
    Wp = np.asarray(W_prop, np.float32)
    bf = np.asarray(b, np.float32)
    bpf = np.asarray(b_prop, np.float32)

    w_main = np.concatenate([Wf, bf[None, :]], axis=0)            # [65, 64]
    w_back = np.concatenate([Wbk, np.zeros((1, D), np.float32)], axis=0)
    w_prop = np.concatenate([Wp, 0.5 * bpf[None, :]], axis=0)

    iota = np.broadcast_to(
        np.tile(np.arange(CH, dtype=np.float32), K), (CH, K * CH)
    ).copy()

    in_maps = []
    for c in range(N_CORES):
        seg = slice(core_starts[c], core_starts[c + 1])
        seg_b = sb[seg]
        seg_a = sa[seg]
        seg_bb = sbb[seg]
        w_local = (seg_b - c * rpc) // CH
        win_starts = np.searchsorted(seg_b, c * rpc + np.arange(nw) * CH)
        slot = np.arange(len(seg_b)) - win_starts[w_local]
        dest = w_local * (K * CH) + slot

        ia = np.zeros(E, np.int32)
        ib = np.zeros(E, np.int32)
        lbr = np.full(E, -1.0, np.float32)
        ia[dest] = ((seg_a - c * rpc) % nt_pad).astype(np.int32)
        ib[dest] = ((seg_bb - c * rpc) % nt_pad).astype(np.int32)
        lbr[dest] = (seg_b - (c * rpc + w_local * CH)).astype(np.float32)

        xT = np.ascontiguousarray(
            np.roll(X_pad, -c * rpc, axis=0).T
        )                                         # [65, nt_pad]

        in_maps.append({
            "xT": xT,
            "w_main": w_main,
            "w_back": w_back,
            "w_prop": w_prop,
            "idxa": np.ascontiguousarray(ia.reshape(ncol, CH).T),
            "idxb": np.ascontiguousarray(ib.reshape(ncol, CH).T),
            "lbr": np.ascontiguousarray(lbr.reshape(ncol, CH).T),
            "iota": iota,
        })

    cfg = dict(N=N, rpc=rpc, nw=nw, nt_pad=nt_pad, K=K)
    return in_maps, cfg


# ----------------------------------------------------------------------------
# device program
# ----------------------------------------------------------------------------

def _build_program(cfg):
    rpc, nw, nt_pad, K = cfg["rpc"], cfg["nw"], cfg["nt_pad"], cfg["K"]
    ncol = nw * K
    ntile = nt_pad // CH                 # 128-row tiles in the table
    TB = 4                               # table tiles per phase-1 batch
    assert ntile % TB == 0 and nw % TB == 0
    nbatch = ntile // TB
    own_batches = nw // TB               # first own_batches cover own slice

    f32 = mybir.dt.float32
    i32 = mybir.dt.int32

    nc = bacc.Bacc("TRN2", target_bir_lowering=False, debug=False,
                   num_devices=N_CORES)

    xT = nc.dram_tensor("xT", [D + 1, nt_pad], f32, kind="ExternalInput").ap()
    w_main = nc.dram_tensor("w_main", [D + 1, D], f32, kind="ExternalInput").ap()
    w_back = nc.dram_tensor("w_back", [D + 1, D], f32, kind="ExternalInput").ap()
    w_prop = nc.dram_tensor("w_prop", [D + 1, D], f32, kind="ExternalInput").ap()
    idxa = nc.dram_tensor("idxa", [CH, ncol], i32, kind="ExternalInput").ap()
    idxb = nc.dram_tensor("idxb", [CH, ncol], i32, kind="ExternalInput").ap()
    lbr = nc.dram_tensor("lbr", [CH, ncol], f32, kind="ExternalInput").ap()
    iota = nc.dram_tensor("iota", [CH, K * CH], f32, kind="ExternalInput").ap()
    table = nc.dram_tensor("table", [nt_pad, D], f32).ap()
    outp = nc.dram_tensor("out", [rpc, D], f32, kind="ExternalOutput").ap()

    with tile.TileContext(nc) as tc:
        with (
            tc.tile_pool(name="wp", bufs=1) as wp,
            tc.tile_pool(name="xp", bufs=3) as xp,
            tc.tile_pool(name="pers", bufs=1) as pers,
            tc.tile_pool(name="stg", bufs=3) as stg,
            tc.tile_pool(name="idxp", bufs=1) as idxp,
            tc.tile_pool(name="gp", bufs=32) as gp,
            tc.tile_pool(name="sp", bufs=2) as sp,
            tc.tile_pool(name="qp", bufs=2) as qp,
            tc.tile_pool(name="op", bufs=3) as op,
            tc.tile_pool(name="ps1", bufs=4, space="PSUM") as ps1,
            tc.tile_pool(name="ps2", bufs=3, space="PSUM") as ps2,
        ):
            wm_t = wp.tile([D + 1, D], f32)
            wb_t = wp.tile([D + 1, D], f32)
            wpr_t = wp.tile([D + 1, D], f32)
            nc.sync.dma_start(out=wm_t[:], in_=w_main[:])
            nc.sync.dma_start(out=wb_t[:], in_=w_back[:])
            nc.sync.dma_start(out=wpr_t[:], in_=w_prop[:])

            ia_t = idxp.tile([CH, ncol], i32)
            ib_t = idxp.tile([CH, ncol], i32)
            lbr_t = idxp.tile([CH, ncol], f32)
            iot_t = idxp.tile([CH, K * CH], f32)
            nc.sync.dma_start(out=ia_t[:], in_=idxa[:])
            nc.sync.dma_start(out=ib_t[:], in_=idxb[:])
            nc.sync.dma_start(out=lbr_t[:], in_=lbr[:])
            nc.sync.dma_start(out=iot_t[:], in_=iota[:])

            xw_s = pers.tile([CH, nw * D], f32)    # XW + b, own slice
            xwb_s = pers.tile([CH, nw * D], f32)   # XW_back, own slice

            # ---------------- phase 1: build table (+ own projections) ----
            for bt in range(nbatch):
                c0 = bt * TB * CH
                xb = xp.tile([D + 1, TB * CH], f32, tag="xb")
                nc.sync.dma_start(out=xb[:], in_=xT[:, c0:c0 + TB * CH])
                ps = ps1.tile([CH, TB * D], f32, tag="psA", space="PSUM")
                for k in range(TB):
                    nc.tensor.matmul(
                        out=ps[:, k * D:(k + 1) * D],
                        lhsT=xb[:, k * CH:(k + 1) * CH],
                        rhs=wpr_t[:],
                        start=True, stop=True,
                    )
                st = stg.tile([CH, TB * D], f32, tag="st")
                nc.scalar.copy(st[:], ps[:])
                nc.sync.dma_start(
                    out=table[bt * TB * CH:(bt + 1) * TB * CH, :].rearrange(
                        "(k p) d -> p k d", p=CH),
                    in_=st[:].rearrange("p (k d) -> p k d", d=D),
                )
                if bt < own_batches:
                    psb = ps1.tile([CH, TB * D], f32, tag="psA", space="PSUM")
                    psc = ps1.tile([CH, TB * D], f32, tag="psA", space="PSUM")
                    for k in range(TB):
                        nc.tensor.matmul(
                            out=psb[:, k * D:(k + 1) * D],
                            lhsT=xb[:, k * CH:(k + 1) * CH],
                            rhs=wm_t[:], start=True, stop=True,
                        )
                        nc.tensor.matmul(
                            out=psc[:, k * D:(k + 1) * D],
                            lhsT=xb[:, k * CH:(k + 1) * CH],
                            rhs=wb_t[:], start=True, stop=True,
                        )
                    nc.scalar.copy(xw_s[:, bt * TB * D:(bt + 1) * TB * D], psb[:])
                    nc.scalar.copy(xwb_s[:, bt * TB * D:(bt + 1) * TB * D], psc[:])

            # ---------------- phase 2: gather / segsum / combine ----------
            for w in range(nw):
                # HW indirect DMA honours only ONE offset per partition:
                # one [128,1]-offset gather per 128-entry chunk, each into its
                # OWN tile so Tile does not serialize them.
                s = sp.tile([CH, K * D], f32, tag="s")
                for k in range(K):
                    ga = gp.tile([CH, D], f32, tag="gak")
                    gb = gp.tile([CH, D], f32, tag="gbk")
                    nc.gpsimd.indirect_dma_start(
                        out=ga[:], out_offset=None,
                        in_=table[:],
                        in_offset=IndirectOffsetOnAxis(
                            ap=ia_t[:, w * K + k:w * K + k + 1], axis=0),
                    )
                    nc.gpsimd.indirect_dma_start(
                        out=gb[:], out_offset=None,
                        in_=table[:],
                        in_offset=IndirectOffsetOnAxis(
                            ap=ib_t[:, w * K + k:w * K + k + 1], axis=0),
                    )
                    nc.vector.tensor_add(s[:, k * D:(k + 1) * D], ga[:], gb[:])
                nc.scalar.activation(s[:], s[:],
                                     mybir.ActivationFunctionType.Relu)
                q = qp.tile([CH, K * CH], f32, tag="q")
                nc.vector.tensor_tensor(
                    out=q[:].rearrange("p (k r) -> p k r", r=CH),
                    in0=lbr_t[:, w * K:(w + 1) * K].to_broadcast([CH, K, CH]),
                    in1=iot_t[:].rearrange("p (k r) -> p k r", r=CH),
                    op=mybir.AluOpType.is_equal,
                )
                cps = ps2.tile([CH, D], f32, tag="cps", space="PSUM")
                for k in range(K):
                    nc.tensor.matmul(
                        out=cps[:],
                        lhsT=q[:, k * CH:(k + 1) * CH],
                        rhs=s[:, k * D:(k + 1) * D],
                        start=(k == 0), stop=(k == K - 1),
                    )
                t2 = op.tile([CH, D], f32, tag="t2")
                nc.vector.tensor_mul(t2[:], xwb_s[:, w * D:(w + 1) * D], cps[:])
                t3 = op.tile([CH, D], f32, tag="t3")
                nc.vector.tensor_add(t3[:], t2[:], xw_s[:, w * D:(w + 1) * D])
                o = op.tile([CH, D], f32, tag="o")
                nc.scalar.activation(o[:], t3[:],
                                     mybir.ActivationFunctionType.Relu)
                nc.sync.dma_start(out=outp[w * CH:(w + 1) * CH, :], in_=o[:])

    nc.compile()
    return nc


# ----------------------------------------------------------------------------
# SPMD runner (device-resident inputs, PJRT under axon)
# ----------------------------------------------------------------------------

class SpmdRunner:
    def __init__(self, nc, n_cores=N_CORES):
        import jax
        from jax.sharding import Mesh, PartitionSpec
        from jax.experimental.shard_map import shard_map
        from concourse.bass2jax import (
            install_neuronx_cc_hook, _bass_exec_p, partition_id_tensor)

        install_neuronx_cc_hook()
        self.jax = jax
        self.nc = nc
        self.n_cores = n_cores
        partition_name = (nc.partition_id_tensor.name
                          if nc.partition_id_tensor else None)

        in_names, out_names, out_avals, zero_shapes = [], [], [], []
        for alloc in nc.m.functions[0].allocations:
            if not isinstance(alloc, mybir.MemoryLocationSet):
                continue
            name = alloc.memorylocations[0].name
            if alloc.kind == "ExternalInput":
                if name != partition_name and (
                        nc.dbg_addr is None or name != nc.dbg_addr.name):
                    in_names.append(name)
            elif alloc.kind == "ExternalOutput":
                out_names.append(name)
                shape = tuple(alloc.tensor_shape)
                dtype = mybir.dt.np(alloc.dtype)
                out_avals.append(jax.core.ShapedArray(shape, dtype))
                zero_shapes.append((shape, dtype))
        self.in_names, self.out_names = in_names, out_names
        self.out_avals, self.zero_shapes = out_avals, zero_shapes
        n_params, n_outs = len(in_names), len(out_names)

        all_in_names = list(in_names) + list(out_names)
        dbg_name = nc.dbg_addr.name if nc.dbg_addr is not None else None
        if dbg_name is not None:
            all_in_names.append(dbg_name)
        if partition_name is not None:
            all_in_names.append(partition_name)

        def _body(*args):
            operands = list(args)
            if dbg_name is not None:
                operands.append(jax.numpy.zeros((1, 2), jax.numpy.uint32))
            if partition_name is not None:
                operands.append(partition_id_tensor())
            outs = _bass_exec_p.bind(
                *operands,
                out_avals=tuple(out_avals),
                in_names=tuple(all_in_names),
                out_names=tuple(out_names),
                lowering_input_output_aliases=(),
                sim_require_finite=True,
                sim_require_nnan=True,
                nc=nc,
            )
            return tuple(outs)

        devices = jax.devices()[:n_cores]
        self.mesh = Mesh(np.asarray(devices), ("core",))
        self.pspec = PartitionSpec("core")
        in_specs = (self.pspec,) * (n_params + n_outs)
        out_specs = (self.pspec,) * n_outs
        self.fn = jax.jit(
            shard_map(_body, mesh=self.mesh, in_specs=in_specs,
                      out_specs=out_specs, check_rep=False),
            donate_argnums=tuple(range(n_params, n_params + n_outs)),
            keep_unused=True,
        )

    def put_inputs(self, in_maps):
        sharding = self.jax.sharding.NamedSharding(self.mesh, self.pspec)
        return [
            self.jax.device_put(
                np.concatenate([np.asarray(m[name]) for m in in_maps], axis=0),
                sharding)
            for name in self.in_names
        ]

    def _zeros(self):
        sharding = self.jax.sharding.NamedSharding(self.mesh, self.pspec)
        return [
            self.jax.device_put(
                np.zeros((self.n_cores * s[0], *s[1:]), d), sharding)
            for (s, d) in self.zero_shapes
        ]

    def run(self, dev_in):
        outs = self.fn(*dev_in, *self._zeros())
        self.jax.block_until_ready(outs)
        return outs

    def results(self, outs):
        res = []
        for c in range(self.n_cores):
            d = {}
            for i, name in enumerate(self.out_names):
                shp = self.out_avals[i].shape
                d[name] = np.asarray(outs[i]).reshape(
                    self.n_cores, *shp)[c]
            res.append(d)
        return res


# ----------------------------------------------------------------------------
# entry point
# ----------------------------------------------------------------------------

def kernel(X, ref_a, ref_b, backref, e_map, v_count, W, W_back, W_prop,
           b, b_prop, **_unused):
    X = np.asarray(X)
    in_maps, cfg = _host_prep(X, ref_a, ref_b, backref,
                              W, W_back, W_prop, b, b_prop)
    key = (cfg["N"], cfg["rpc"], cfg["K"])
    if key not in _cache:
        nc = _build_program(cfg)
        _cache[key] = SpmdRunner(nc)
    runner = _cache[key]
    dev_in = runner.put_inputs(in_maps)
    outs = runner.run(dev_in)
    res = runner.results(outs)
    full = np.concatenate([res[c]["out"] for c in range(N_CORES)], axis=0)
    return full[:cfg["N"]].astype(np.float32)
